# revision 1
# baseline (speedup 1.0000x reference)
"""DeepSeekMoE forward on 8 TRN2 cores — gathered expert-parallel version.

Sharding as kernel.py (routed expert c -> core c, shared experts 8-way
H-sliced, router replicated). The routed FFN runs only on the ~N*topk/E
tokens routed to this core's expert:

  - compaction: top-2 mask -> per-tile counts + prefix sums (triangular
    fp32 matmuls, exact for small ints) -> compact slot per selected
    token (unselected -> out-of-range slot `cap`, which never matches)
  - slot tables WITHOUT indirect scatters (they are descriptor-rate
    bound): one-hot permutation tiles P[t,s] = (pos[t]==s) built by DVE
    compares, then tiny fp32 matmuls P.T @ [tile_id, part_id, gate]
    produce slot-ordered (token id, gate) tables in SBUF
  - indirect-DMA row gather of x for the selected tokens, PE-transpose
    into xgT [D, slot]
  - routed FFN (fp32r) on `cap` slots, gated by gathered gate (empty
    slots have gate 0); compact output rows [cap, D]
  - shared experts run densely over all tokens (emitted FIRST so the
    scheduler overlaps them with the whole compaction pipeline)
  - host: out = x + sum_c shared_c; out[idx_c[:cnt_c]] += routed_c[:cnt_c]
"""

import sys
from contextlib import ExitStack

if "/opt/trn_rl_repo" not in sys.path:
    sys.path.insert(0, "/opt/trn_rl_repo")

import numpy as np

import concourse.bass as bass
import concourse.mybir as mybir
import concourse.tile as tile
from concourse import bacc
from concourse.bass import IndirectOffsetOnAxis
from concourse.bass_utils import run_bass_kernel_spmd

F32 = mybir.dt.float32
F32R = mybir.dt.float32r
I32 = mybir.dt.int32
AF = mybir.ActivationFunctionType
OP = mybir.AluOpType
AX = mybir.AxisListType

N_CORES = 8
D = 1024
H = 4096
HS = 1024
E = 8
P = 128

TOK_BLOCK = 1024   # shared-expert token blocking
H_BLOCK_S = 1024   # shared-expert weight blocking
H_BLOCK_R = 512    # routed-expert weight blocking


def _chunks(n, step=512):
    out, o = [], 0
    while o < n:
        out.append((o, min(step, n - o)))
        o += step
    return out


def build_nc(n_tok: int, cap: int, num_devices: int = N_CORES):
    assert n_tok % TOK_BLOCK == 0 and cap % P == 0
    nc = bacc.Bacc("TRN2", target_bir_lowering=False, debug=False,
                   num_devices=num_devices)
    aps = {}

    def dram(name, shape, dt, kind="ExternalInput"):
        aps[name] = nc.dram_tensor(name, shape, dt, kind=kind).ap()

    TT = n_tok // P
    dram("xT", [D, n_tok], F32R)
    dram("xrows", [n_tok, D], F32)
    dram("rn", [n_tok, E], F32)
    dram("wrn", [D, 2 * E], F32)
    dram("brbn", [1, 2 * E], F32)
    dram("esel", [P, E], F32)
    dram("ones32", [1, P], F32)
    dram("onescol", [P, 1], F32)
    dram("triu128", [P, P], F32)     # [j,i]=1 if j<i
    dram("triu32", [TT, TT], F32)
    dram("iotaf", [P, P], F32)       # [p,s] = s
    dram("ighl", [P, TT, 3], F32)    # [:,tt,0]=tt, [:,tt,1]=p, [:,tt,2]=0
    dram("id128", [P, P], F32)
    dram("w1", [D, H], F32R)
    dram("b1", [P, H // P], F32)
    dram("w2", [H, D], F32R)
    dram("sw1", [D, HS], F32R)
    dram("sb1", [P, HS // P], F32)
    dram("sw2", [HS, D], F32R)
    dram("out_sh", [n_tok, D], F32, kind="ExternalOutput")
    dram("out_rt", [cap, D], F32, kind="ExternalOutput")
    dram("idx_t", [cap, 1], I32, kind="ExternalOutput")
    dram("gate_o", [cap, 1], F32, kind="ExternalOutput")
    dram("cnt_t", [1, 1], F32, kind="ExternalOutput")

    with tile.TileContext(nc) as tc:
        with ExitStack() as es:
            _emit(es, tc, nc, aps, n_tok, cap)
    nc.compile()
    return nc


def _emit(es, tc, nc, aps, n_tok, cap):
    TT = n_tok // P
    DS = D // P
    NTC = cap // P

    A = type("A", (), aps)

    cpool = es.enter_context(tc.tile_pool(name="const", bufs=1))
    rpool = es.enter_context(tc.tile_pool(name="router", bufs=2))
    spool = es.enter_context(tc.tile_pool(name="rscratch", bufs=1))
    gpool = es.enter_context(tc.tile_pool(name="gather", bufs=2))
    rpsum = es.enter_context(tc.tile_pool(name="rpsum", bufs=2, space="PSUM"))
    xpool = es.enter_context(tc.tile_pool(name="xb", bufs=1))
    w1pool = es.enter_context(tc.tile_pool(name="w1b", bufs=1))
    w2pool = es.enter_context(tc.tile_pool(name="w2b", bufs=1))
    hpool = es.enter_context(tc.tile_pool(name="hT", bufs=1))
    ypool = es.enter_context(tc.tile_pool(name="yacc", bufs=1))
    psum = es.enter_context(tc.tile_pool(name="psum", bufs=6, space="PSUM"))

    def ctile(shape, dt, name):
        return cpool.tile(shape, dt, name=name, tag=name)

    def stile(shape, name, dt=F32, bufs=None):
        return spool.tile(shape, dt, name=name, tag=name, bufs=bufs)

    def rps(shape, name):
        return rpsum.tile(shape, F32, name=name, tag="rps")

    def load_const(name, shape, dt):
        t = ctile(shape, dt, name + "_sb")
        nc.sync.dma_start(t[:], aps[name][:])
        return t

    # ---- constants ----
    wrn_sb = ctile([P, DS, 2 * E], F32, "wrn_sb")
    nc.sync.dma_start(wrn_sb[:], A.wrn.rearrange("(ds p) e -> p ds e", p=P))
    brbn_sb = load_const("brbn", [1, 2 * E], F32)
    esel_sb = load_const("esel", [P, E], F32)
    ones32_sb = load_const("ones32", [1, P], F32)
    onescol_sb = load_const("onescol", [P, 1], F32)
    triu128_sb = load_const("triu128", [P, P], F32)
    triu32_sb = load_const("triu32", [TT, TT], F32)
    iotaf_sb = load_const("iotaf", [P, P], F32)
    id_sb = load_const("id128", [P, P], F32)
    b1_sb = load_const("b1", [P, H // P], F32)
    sb1_sb = load_const("sb1", [P, HS // P], F32)

    # ---- FFN helpers (fp32r) ----
    # Chunk-inner loops keep the stationary operand loaded across all
    # output chunks (PE pulls LDWEIGHTS ahead only between weight changes),
    # accumulating into several PSUM banks concurrently.
    def gemm1(xsrc, nb, w1b_t, hT_t, bias_sb, bias_off, nsub):
        ch = _chunks(nb)
        for hs in range(nsub):
            pss = [psum.tile([P, 512], F32, name="ps_g1", tag="ps")
                   for _ in ch]
            for ds in range(DS):
                for ci, (no, nw) in enumerate(ch):
                    nc.tensor.matmul(
                        pss[ci][:, :nw], w1b_t[:, ds, hs * P:(hs + 1) * P],
                        xsrc[:, ds, no:no + nw],
                        start=(ds == 0), stop=(ds == DS - 1))
            for ci, (no, nw) in enumerate(ch):
                nc.scalar.activation(
                    hT_t[:, hs, no:no + nw], pss[ci][:, :nw], AF.Relu,
                    bias=bias_sb[:, bias_off + hs:bias_off + hs + 1])

    def gemm2(y_acc, hT_t, w2b_t, nt, nsub, first):
        ch = _chunks(D)
        for tt in range(nt):
            pss = [psum.tile([P, 512], F32, name="ps_g2", tag="ps")
                   for _ in ch]
            for hs in range(nsub):
                for ci, (do, dw) in enumerate(ch):
                    nc.tensor.matmul(
                        pss[ci][:, :dw], hT_t[:, hs, tt * P:(tt + 1) * P],
                        w2b_t[:, hs, do:do + dw],
                        start=(hs == 0), stop=(hs == nsub - 1))
            for ci, (do, dw) in enumerate(ch):
                ys = y_acc[:, tt, do:do + dw]
                if first:
                    nc.scalar.activation(ys, pss[ci][:, :dw], AF.Copy)
                else:
                    nc.vector.tensor_add(ys, ys, pss[ci][:, :dw])

    # ---- router phase (fp32) ----
    lgnl = stile([P, TT, 2 * E], "lgnl")
    xT32 = A.xT.bitcast(F32)
    for tt in range(TT):
        xt_r = rpool.tile([P, DS, P], F32, name="xt_r")
        nc.gpsimd.dma_start(
            xt_r[:],
            xT32[:, tt * P:(tt + 1) * P].rearrange("(ds p) t -> p ds t", p=P))
        ps = rps([P, 2 * E], "ps_r")
        for ds in range(DS):
            nc.tensor.matmul(ps[:], xt_r[:, ds, :], wrn_sb[:, ds, :],
                             start=(ds == 0), stop=False)
        nc.tensor.matmul(ps[:], ones32_sb[:1, :], brbn_sb[:1, :],
                         start=False, stop=True)
        nc.scalar.activation(lgnl[:, tt, :], ps[:], AF.Copy)

    gate = stile([P, TT], "gate")
    mask = stile([P, TT], "mask")
    RC = 8
    for c0 in range(0, TT, RC):
        lg = lgnl[:, c0:c0 + RC, 0:E]
        nl = lgnl[:, c0:c0 + RC, E:2 * E]
        shp = [P, RC, E]

        e0 = stile(shp, "e0"); nc.scalar.activation(e0[:], nl, AF.Exp)
        l0 = stile(shp, "l0"); nc.scalar.activation(l0[:], e0[:], AF.Ln)
        r0 = stile(shp, "r0"); nc.vector.tensor_sub(r0[:], nl, l0[:])
        t0 = stile(shp, "t0"); nc.vector.tensor_mul(t0[:], e0[:], r0[:])
        ee = stile(shp, "ee"); nc.vector.tensor_add(ee[:], e0[:], t0[:])
        uu = stile(shp, "uu"); nc.vector.tensor_scalar_add(uu[:], ee[:], 1.0)
        s0 = stile(shp, "s0"); nc.scalar.activation(s0[:], uu[:], AF.Ln)
        e1 = stile(shp, "e1"); nc.scalar.activation(e1[:], s0[:], AF.Exp)
        l1 = stile(shp, "l1"); nc.scalar.activation(l1[:], e1[:], AF.Ln)
        r1 = stile(shp, "r1"); nc.vector.tensor_sub(r1[:], s0[:], l1[:])
        t1 = stile(shp, "t1"); nc.vector.tensor_mul(t1[:], e1[:], r1[:])
        e1p = stile(shp, "e1p"); nc.vector.tensor_add(e1p[:], e1[:], t1[:])
        re1 = stile(shp, "re1"); nc.vector.reciprocal(re1[:], e1p[:])
        dd = stile(shp, "dd"); nc.vector.tensor_mul(dd[:], uu[:], re1[:])
        dm = stile(shp, "dm"); nc.vector.tensor_scalar_add(dm[:], dd[:], -1.0)
        sp = stile(shp, "sp"); nc.vector.tensor_add(sp[:], s0[:], dm[:])

        rn_sb = stile(shp, "rn_sb")
        nc.gpsimd.dma_start(
            rn_sb[:],
            A.rn[c0 * P:(c0 + RC) * P, :].rearrange("(t p) e -> p t e", p=P))
        noise = stile(shp, "noise"); nc.vector.tensor_mul(noise[:], rn_sb[:], sp[:])
        noisy = stile(shp, "noisy"); nc.vector.tensor_add(noisy[:], lg, noise[:])

        m1 = stile([P, RC], "m1")
        nc.vector.tensor_reduce(m1[:], noisy[:], axis=AX.X, op=OP.max)
        m1b = m1[:, :, None].broadcast_to(shp)
        eq = stile(shp, "eq")
        nc.vector.tensor_tensor(eq[:], noisy[:], m1b, op=OP.is_equal)
        big = stile(shp, "big"); nc.vector.tensor_scalar_mul(big[:], eq[:], 1e30)
        noisy2 = stile(shp, "noisy2"); nc.vector.tensor_sub(noisy2[:], noisy[:], big[:])
        m2 = stile([P, RC], "m2")
        nc.vector.tensor_reduce(m2[:], noisy2[:], axis=AX.X, op=OP.max)
        m2b = m2[:, :, None].broadcast_to(shp)
        ge = stile(shp, "ge")
        nc.vector.tensor_tensor(ge[:], noisy[:], m2b, op=OP.is_ge)
        shd = stile(shp, "shd"); nc.vector.tensor_sub(shd[:], noisy[:], m1b)
        ex = stile(shp, "ex"); nc.scalar.activation(ex[:], shd[:], AF.Exp)
        gg = stile(shp, "gg"); nc.vector.tensor_mul(gg[:], ex[:], ge[:])
        den = stile([P, RC], "den")
        nc.vector.tensor_reduce(den[:], gg[:], axis=AX.X, op=OP.add)
        rden = stile([P, RC], "rden")
        nc.vector.reciprocal(rden[:], den[:])
        gate8 = stile(shp, "gate8")
        nc.vector.tensor_tensor(gate8[:], gg[:],
                                rden[:, :, None].broadcast_to(shp), op=OP.mult)
        gsel = stile(shp, "gsel")
        nc.vector.tensor_tensor(gsel[:], gate8[:],
                                esel_sb[:, None, :].broadcast_to(shp), op=OP.mult)
        nc.vector.tensor_reduce(gate[:, c0:c0 + RC], gsel[:], axis=AX.X, op=OP.add)
        msel = stile(shp, "msel")
        nc.vector.tensor_tensor(msel[:], ge[:],
                                esel_sb[:, None, :].broadcast_to(shp), op=OP.mult)
        nc.vector.tensor_reduce(mask[:, c0:c0 + RC], msel[:], axis=AX.X, op=OP.add)

    # ---- shared experts (independent of routing -> overlaps compaction;
    # the last block is emitted after the routed FFN so pool-slot rotation
    # hides both phase-transition weight loads) ----
    NB = TOK_BLOCK
    NT = NB // P

    def shared_block(b):
        tok0 = b * NB
        xb = xpool.tile([P, DS, NB], F32R, name="xb", tag="xb")
        for (no_, nw_) in (_chunks(NB) if b == 0 else [(0, NB)]):
            nc.sync.dma_start(
                xb[:, :, no_:no_ + nw_],
                A.xT[:, tok0 + no_:tok0 + no_ + nw_].rearrange(
                    "(ds p) t -> p ds t", p=P))
        y_s = ypool.tile([P, NT, D], F32, name="y_s", tag="y_acc")
        HSUB_S = H_BLOCK_S // P
        for hb in range(HS // H_BLOCK_S):
            sw1b = w1pool.tile([P, DS, H_BLOCK_S], F32R, name="sw1b", tag="w1b")
            for (ho_, hw_) in (_chunks(H_BLOCK_S, 128) if b == 0 else [(0, H_BLOCK_S)]):
                nc.sync.dma_start(
                    sw1b[:, :, ho_:ho_ + hw_],
                    A.sw1[:, hb * H_BLOCK_S + ho_:
                          hb * H_BLOCK_S + ho_ + hw_].rearrange(
                        "(ds p) h -> p ds h", p=P))
            hTs = hpool.tile([P, HSUB_S, NB], F32R, name="hTs", tag="hTb")
            gemm1(xb, NB, sw1b, hTs, sb1_sb, hb * HSUB_S, HSUB_S)
            sw2b = w2pool.tile([P, HSUB_S, D], F32R, name="sw2b", tag="w2b")
            nc.sync.dma_start(
                sw2b[:], A.sw2[hb * H_BLOCK_S:(hb + 1) * H_BLOCK_S, :].rearrange(
                    "(hs p) d -> p hs d", p=P))
            gemm2(y_s, hTs, sw2b, NT, HSUB_S, first=(hb == 0))
        for tt in range(NT):
            nc.sync.dma_start(A.out_sh[tok0 + tt * P:tok0 + (tt + 1) * P, :],
                              y_s[:, tt, :])

    for b in range(n_tok // NB):
        shared_block(b)

    # ---- compaction: slot = prefix(mask); unselected -> `cap` (no slot) --
    cntp = rps([TT, 1], "cntp")
    nc.tensor.matmul(cntp[:], mask[:], onescol_sb[:], start=True, stop=True)
    cnt_sb = stile([TT, 1], "cnt_sb")
    nc.scalar.activation(cnt_sb[:], cntp[:], AF.Copy)
    ecsp = rps([1, TT], "ecsp")
    nc.tensor.matmul(ecsp[:], cnt_sb[:], triu32_sb[:], start=True, stop=True)
    ecs_row = stile([1, TT], "ecs_row")
    nc.scalar.activation(ecs_row[:], ecsp[:], AF.Copy)
    totp = rps([1, 1], "totp")
    nc.tensor.matmul(totp[:], cnt_sb[:], onescol_sb[:TT, :], start=True, stop=True)
    tot_sb = stile([1, 1], "tot_sb")
    nc.scalar.activation(tot_sb[:], totp[:], AF.Copy)
    nc.sync.dma_start(A.cnt_t[:], tot_sb[:])

    posp = rps([P, TT], "posp")
    nc.tensor.matmul(posp[:], triu128_sb[:], mask[:], start=True, stop=False)
    nc.tensor.matmul(posp[:], ones32_sb[:1, :], ecs_row[:1, :],
                     start=False, stop=True)
    pos = stile([P, TT], "pos")
    nc.scalar.activation(pos[:], posp[:], AF.Copy)
    # pos_final = pos*mask + (1-mask)*cap
    pm_a = stile([P, TT], "pm_a"); nc.vector.tensor_mul(pm_a[:], pos[:], mask[:])
    pm_b = stile([P, TT], "pm_b")
    nc.vector.tensor_scalar_mul(pm_b[:], mask[:], float(cap))
    pm_c = stile([P, TT], "pm_c"); nc.vector.tensor_sub(pm_c[:], pm_a[:], pm_b[:])
    pm = stile([P, TT], "pm")
    nc.vector.tensor_scalar_add(pm[:], pm_c[:], float(cap))

    # ---- slot tables via one-hot permutation matmuls (fp32, exact) ----
    # igr[:, tt, :] = (tile_id, part_id, gate_tt)
    igr = stile([P, TT, 3], "igr")
    nc.sync.dma_start(igr[:], A.ighl[:])
    nc.vector.tensor_copy(igr[:, :, 2], gate[:])
    ig_sb = stile([P, NTC, 3], "ig_sb")
    for st in range(NTC):
        ps_ig = rps([P, 3], "ps_ig")
        for tt in range(TT):
            pshift = stile([P, 1], "pshift", bufs=3)
            nc.vector.tensor_scalar_add(pshift[:], pm[:, tt:tt + 1],
                                        -float(st * P))
            ptile = stile([P, P], "ptile", bufs=2)
            nc.vector.tensor_scalar(ptile[:], iotaf_sb[:], pshift[:], None,
                                    op0=OP.is_equal)
            nc.tensor.matmul(ps_ig[:], ptile[:], igr[:, tt, :],
                             start=(tt == 0), stop=(tt == TT - 1))
        nc.scalar.activation(ig_sb[:, st, :], ps_ig[:], AF.Copy)

    # idx = hi*128 + lo ; gate_g = col 2
    idxf = stile([P, NTC], "idxf")
    nc.vector.tensor_scalar(idxf[:], ig_sb[:, :, 0], float(P), None, op0=OP.mult)
    nc.vector.tensor_add(idxf[:], idxf[:], ig_sb[:, :, 1])
    idx_g = stile([P, NTC], "idx_g", I32)
    nc.vector.tensor_copy(idx_g[:], idxf[:])
    gate_g = stile([P, NTC], "gate_g")
    nc.vector.tensor_copy(gate_g[:], ig_sb[:, :, 2])
    nc.sync.dma_start(A.idx_t.rearrange("(st p) o -> p (st o)", p=P), idx_g[:])
    nc.sync.dma_start(A.gate_o.rearrange("(st p) o -> p (st o)", p=P), gate_g[:])

    # ---- gather x rows for selected tokens, transpose to xgT [d, slot] ----
    xgT = xpool.tile([P, DS, cap], F32R, name="xgT", tag="xb")
    for st in range(NTC):
        xg = gpool.tile([P, D], F32, name="xg", tag="xg")
        nc.gpsimd.indirect_dma_start(
            out=xg[:], in_=A.xrows[:],
            in_offset=IndirectOffsetOnAxis(ap=idx_g[:, st:st + 1], axis=0),
            out_offset=None)
        for dp in range(DS):
            tps = rps([P, P], "tps")
            nc.tensor.transpose(tps[:], xg[:, dp * P:(dp + 1) * P], id_sb[:])
            nc.scalar.activation(xgT[:, dp, st * P:(st + 1) * P], tps[:], AF.Copy)

    # ---- routed FFN on gathered tokens ----
    y_acc = ypool.tile([P, NTC, D], F32, name="y_acc", tag="y_acc")
    HSUB_R = H_BLOCK_R // P
    for hb in range(H // H_BLOCK_R):
        w1b = w1pool.tile([P, DS, H_BLOCK_R], F32R, name="w1b", tag="w1b")
        for hs_ in range(H_BLOCK_R // P):
            nc.sync.dma_start(
                w1b[:, :, hs_ * P:(hs_ + 1) * P],
                A.w1[:, hb * H_BLOCK_R + hs_ * P:
                     hb * H_BLOCK_R + (hs_ + 1) * P].rearrange(
                    "(ds p) h -> p ds h", p=P))
        hTb = hpool.tile([P, HSUB_R, cap], F32R, name="hTb", tag="hTb")
        gemm1(xgT, cap, w1b, hTb, b1_sb, hb * HSUB_R, HSUB_R)
        w2b = w2pool.tile([P, HSUB_R, D], F32R, name="w2b", tag="w2b")
        nc.sync.dma_start(
            w2b[:], A.w2[hb * H_BLOCK_R:(hb + 1) * H_BLOCK_R, :].rearrange(
                "(hs p) d -> p hs d", p=P))
        gemm2(y_acc, hTb, w2b, NTC, HSUB_R, first=(hb == 0))
    for tt in range(NTC):
        nc.vector.tensor_scalar_mul(y_acc[:, tt, :], y_acc[:, tt, :],
                                    gate_g[:, tt:tt + 1])
        nc.sync.dma_start(A.out_rt[tt * P:(tt + 1) * P, :], y_acc[:, tt, :])


# ---------------- host side ----------------

_NC_CACHE = {}
CAP = 1152


def _get_nc(n_tok, cap):
    key = (n_tok, cap)
    if key not in _NC_CACHE:
        _NC_CACHE[key] = build_nc(n_tok, cap)
    return _NC_CACHE[key]


def make_in_maps(n_tok, cap, x, router_noise, Wr, br, Wn, bn, rW1, rb1, rW2,
                 rb2, sW1, sb1, sW2, sb2):
    TT = n_tok // P
    xf = np.ascontiguousarray(x.reshape(n_tok, D))
    xT = np.ascontiguousarray(xf.T)
    rnf = np.ascontiguousarray(router_noise.reshape(n_tok, E)).astype(np.float32)
    wrn = np.ascontiguousarray(np.concatenate([Wr, Wn], axis=1)).astype(np.float32)
    brbn = np.concatenate([br, bn]).reshape(1, 2 * E).astype(np.float32)
    ones = np.ones((1, P), np.float32)
    ighl = np.zeros((P, TT, 3), np.float32)
    ighl[:, :, 0] = np.arange(TT)[None, :]
    ighl[:, :, 1] = np.arange(P)[:, None]

    in_maps = []
    for c in range(N_CORES):
        se, hsl = c // 4, (c % 4) * HS
        esel = np.zeros((P, E), np.float32)
        esel[:, c] = 1.0
        in_maps.append({
            "xT": xT,
            "xrows": xf,
            "rn": rnf,
            "wrn": wrn,
            "brbn": brbn,
            "esel": esel,
            "ones32": ones,
            "onescol": np.ones((P, 1), np.float32),
            "triu128": np.triu(np.ones((P, P), np.float32), 1),
            "triu32": np.triu(np.ones((TT, TT), np.float32), 1),
            "iotaf": np.tile(np.arange(P, dtype=np.float32)[None, :], (P, 1)),
            "ighl": ighl,
            "id128": np.eye(P, dtype=np.float32),
            "w1": np.ascontiguousarray(rW1[c]),
            "b1": np.ascontiguousarray(rb1[c].reshape(H // P, P).T),
            "w2": np.ascontiguousarray(rW2[c]),
            "sw1": np.ascontiguousarray(sW1[se][:, hsl:hsl + HS]),
            "sb1": np.ascontiguousarray(
                sb1[se][hsl:hsl + HS].reshape(HS // P, P).T),
            "sw2": np.ascontiguousarray(sW2[se][hsl:hsl + HS, :]),
        })
    return in_maps


def combine(x, results, n_tok, cap, rb2, sb2):
    acc = x.reshape(n_tok, D).astype(np.float32).copy()
    acc += sb2.sum(axis=0).astype(np.float32)
    for c in range(N_CORES):
        acc += results[c]["out_sh"]
    for c in range(N_CORES):
        n = int(round(float(results[c]["cnt_t"][0, 0])))
        assert n <= cap, f"core {c}: count {n} exceeds capacity {cap}"
        idx = results[c]["idx_t"][:n, 0]
        g = results[c]["gate_o"][:n]
        acc[idx] += results[c]["out_rt"][:n] + g * rb2[c][None, :]
    return acc


def kernel(x, router_noise, topk, Wr, br, Wn, bn, rW1, rb1, rW2, rb2,
           sW1, sb1, sW2, sb2, _trace=False):
    assert int(topk) == 2
    x = np.asarray(x, np.float32)
    B, T, Dx = x.shape
    n_tok = B * T
    nc = _get_nc(n_tok, CAP)
    in_maps = make_in_maps(
        n_tok, CAP, x, np.asarray(router_noise, np.float32),
        np.asarray(Wr, np.float32), np.asarray(br, np.float32),
        np.asarray(Wn, np.float32), np.asarray(bn, np.float32),
        np.asarray(rW1, np.float32), np.asarray(rb1, np.float32),
        np.asarray(rW2, np.float32), np.asarray(rb2, np.float32),
        np.asarray(sW1, np.float32), np.asarray(sb1, np.float32),
        np.asarray(sW2, np.float32), np.asarray(sb2, np.float32))
    res = run_bass_kernel_spmd(nc, in_maps, core_ids=list(range(N_CORES)),
                               trace=_trace)
    out = combine(x, res.results, n_tok, CAP,
                  np.asarray(rb2, np.float32),
                  np.asarray(sb2, np.float32)).reshape(B, T, Dx)
    if _trace:
        return out, res
    return out



# revision 10
# speedup vs baseline: 1.2014x; 1.2014x over previous
"""DeepSeekMoE forward on 8 TRN2 cores — v2.

Sharding: routed expert c -> core c; shared experts 8-way H-sliced;
router replicated. vs v1:

  - slot tables built by indirect-DMA *scatter* (pos -> DRAM -> readback)
    instead of 288 one-hot permutation matmuls (-135us PE, -81us DVE);
    unselected tokens get pos=cap and are dropped by the DMA bounds
    check, junk in empty slots is ignored by the host (reads cnt_t).
  - router matmul flipped: stationary = Wr|Wn [128,16] fp32 blocks,
    moving = x 256-token chunks, accumulate over D in PSUM [16, tok];
    bias fused in the PSUM->SBUF copy; PE-transpose back to [tok, e].
    fp32 kept exactly: min top-2/3 noisy-logit gap is 2.2e-4.
  - all FFN matmuls in bf16 (max-err contribution ~1e-2 vs budget 0.15),
    fp32 PSUM accumulation; weights cast on host.
  - routed FFN in two token-halves (640+512 of cap=1152): hT resident
    per half, gemm2 accumulates all 32 h-blocks in PSUM (no vector
    adds), gate scaling fused into the PSUM->staging copy, staging
    DMA'd straight to out_rt.
  - moving chunks all >=256 (no 128-wide matmul stragglers).
  - host: out = x + sum_c shared_c; out[idx_c[:cnt]] += routed_c[:cnt]
"""

import sys
from contextlib import ExitStack

if "/opt/trn_rl_repo" not in sys.path:
    sys.path.insert(0, "/opt/trn_rl_repo")

import numpy as np
import ml_dtypes

import concourse.bass as bass
import concourse.mybir as mybir
import concourse.tile as tile
from concourse import bacc
from concourse.bass import IndirectOffsetOnAxis
from concourse.bass_utils import run_bass_kernel_spmd

F32 = mybir.dt.float32
F32R = mybir.dt.float32r
BF16 = mybir.dt.bfloat16
I32 = mybir.dt.int32
AF = mybir.ActivationFunctionType
OP = mybir.AluOpType
AX = mybir.AxisListType

N_CORES = 8
D = 1024
H = 4096
HS = 1024
E = 8
P = 128

NB = 1024          # shared-expert token block
RXC = 128          # router moving-chunk (tokens)
CAP = 1152
HALVES = [(0, 5), (5, 4)]   # (start st-tile, n st-tiles) of cap/128=9


def _chunks(n, step=512):
    out, o = [], 0
    while o < n:
        out.append((o, min(step, n - o)))
        o += step
    return out


def build_nc(n_tok: int, cap: int, num_devices: int = N_CORES):
    assert n_tok % NB == 0 and cap % P == 0
    nc = bacc.Bacc("TRN2", target_bir_lowering=False, debug=False,
                   num_devices=num_devices)
    aps = {}

    def dram(name, shape, dt, kind="ExternalInput"):
        aps[name] = nc.dram_tensor(name, shape, dt, kind=kind).ap()

    TT = n_tok // P
    dram("xT", [D, n_tok], F32)          # router moving operand
    dram("xTb", [D, n_tok], BF16)        # shared-expert moving operand
    dram("xrowsb", [n_tok, D], BF16)     # gather source
    dram("rn", [n_tok, E], F32)
    dram("wrn", [D, 2 * E], F32)
    dram("brbn", [2 * E, 1], F32)
    dram("esel", [P, E], F32)
    dram("ones32", [1, P], F32)
    dram("onescol", [P, 1], F32)
    dram("triu128", [P, P], F32)         # [j,i]=1 if j<i
    dram("triu32", [TT, TT], F32)
    dram("id16", [16, 16], F32)
    dram("id128b", [P, P], BF16)
    dram("payt", [P, TT, 2], F32)        # [:,tt,0]=tt*128+p, [:,tt,1]=0
    dram("w1", [D, H], BF16)
    dram("b1", [P, H // P], F32)
    dram("w2", [H, D], BF16)
    dram("sw1", [D, HS], BF16)
    dram("sb1", [P, HS // P], F32)
    dram("sw2", [HS, D], BF16)
    dram("scat", [cap, 2], F32, kind="ExternalOutput")
    dram("out_sh", [n_tok, D], F32, kind="ExternalOutput")
    dram("out_rt", [cap, D], F32, kind="ExternalOutput")
    dram("cnt_t", [1, 1], F32, kind="ExternalOutput")

    with tile.TileContext(nc) as tc:
        with ExitStack() as es:
            _emit(es, tc, nc, aps, n_tok, cap)
    nc.compile()
    return nc


def _emit(es, tc, nc, aps, n_tok, cap):
    TT = n_tok // P
    DS = D // P
    NTC = cap // P

    A = type("A", (), aps)

    cpool = es.enter_context(tc.tile_pool(name="const", bufs=1))
    rxpool = es.enter_context(tc.tile_pool(name="rx", bufs=2))
    spool = es.enter_context(tc.tile_pool(name="rscratch", bufs=1))
    gpool = es.enter_context(tc.tile_pool(name="gather", bufs=2))
    rpsum = es.enter_context(tc.tile_pool(name="rpsum", bufs=2, space="PSUM"))
    xpool = es.enter_context(tc.tile_pool(name="xb", bufs=1))
    xgpool = es.enter_context(tc.tile_pool(name="xgt", bufs=1))
    w1pool = es.enter_context(tc.tile_pool(name="w1b", bufs=2))
    swpool = es.enter_context(tc.tile_pool(name="swb", bufs=1))
    w2rpool = es.enter_context(tc.tile_pool(name="w2r", bufs=1))
    hspool = es.enter_context(tc.tile_pool(name="hTs", bufs=1))
    hrpool = es.enter_context(tc.tile_pool(name="hTr", bufs=1))
    ypool = es.enter_context(tc.tile_pool(name="ystg", bufs=3))
    psum = es.enter_context(tc.tile_pool(name="psum", bufs=6, space="PSUM"))

    def ctile(shape, dt, name):
        return cpool.tile(shape, dt, name=name, tag=name)

    def stile(shape, name, dt=F32, bufs=None):
        return spool.tile(shape, dt, name=name, tag=name, bufs=bufs)

    def rps(shape, name, dt=F32):
        return rpsum.tile(shape, dt, name=name, tag="rps")

    def load_const(name, shape, dt):
        t = ctile(shape, dt, name + "_sb")
        nc.sync.dma_start(t[:], aps[name][:])
        return t

    # ---- constants ----
    wrn_sb = ctile([P, DS, 2 * E], F32, "wrn_sb")
    nc.sync.dma_start(wrn_sb[:], A.wrn.rearrange("(ds p) e -> p ds e", p=P))
    brbn_sb = load_const("brbn", [2 * E, 1], F32)
    esel_sb = load_const("esel", [P, E], F32)
    ones32_sb = load_const("ones32", [1, P], F32)
    onescol_sb = load_const("onescol", [P, 1], F32)
    triu128_sb = load_const("triu128", [P, P], F32)
    triu32_sb = load_const("triu32", [TT, TT], F32)
    id16_sb = load_const("id16", [16, 16], F32)
    id128b_sb = load_const("id128b", [P, P], BF16)
    b1_sb = load_const("b1", [P, H // P], F32)
    sb1_sb = load_const("sb1", [P, HS // P], F32)

    # ---- router phase: noisy = [logits | pre-softplus] in [tok, 16] ----
    # flipped: stationary = wrn d-block [128,16] fp32 (exact), moving =
    # x token-chunks; accumulate over D in PSUM [16, chunk]; bias in the
    # copy out; PE-transpose each [16,128] to [128,16] of lgnl.
    lgnl = stile([P, TT, 2 * E], "lgnl")
    for rb in range(n_tok // RXC):
        xr = rxpool.tile([P, DS, RXC], F32, name="xr", tag="xr")
        nc.gpsimd.dma_start(
            xr[:],
            A.xT[:, rb * RXC:(rb + 1) * RXC].rearrange(
                "(ds p) t -> p ds t", p=P))
        ps_r = rps([2 * E, RXC], "ps_r")
        for ds in range(DS):
            nc.tensor.matmul(ps_r[:], wrn_sb[:, ds, :], xr[:, ds, :],
                             start=(ds == 0), stop=(ds == DS - 1))
        lgch = stile([2 * E, RXC], "lgch", bufs=2)
        nc.scalar.activation(lgch[:], ps_r[:], AF.Identity, bias=brbn_sb[:])
        for k in range(RXC // P):
            tps = rps([P, 2 * E], "tps_r")
            nc.tensor.transpose(tps[:], lgch[:, k * P:(k + 1) * P], id16_sb[:])
            tt = (rb * RXC) // P + k
            nc.scalar.activation(lgnl[:, tt, :], tps[:], AF.Copy)

    # ---- router chain: noisy top-2 -> gate [P,TT], mask [P,TT] ----
    gate = stile([P, TT], "gate")
    mask = stile([P, TT], "mask")
    RC = 8
    for c0 in range(0, TT, RC):
        lg = lgnl[:, c0:c0 + RC, 0:E]
        nl = lgnl[:, c0:c0 + RC, E:2 * E]
        shp = [P, RC, E]

        e0 = stile(shp, "e0"); nc.scalar.activation(e0[:], nl, AF.Exp)
        l0 = stile(shp, "l0"); nc.scalar.activation(l0[:], e0[:], AF.Ln)
        r0 = stile(shp, "r0"); nc.vector.tensor_sub(r0[:], nl, l0[:])
        t0 = stile(shp, "t0"); nc.vector.tensor_mul(t0[:], e0[:], r0[:])
        ee = stile(shp, "ee"); nc.vector.tensor_add(ee[:], e0[:], t0[:])
        uu = stile(shp, "uu"); nc.vector.tensor_scalar_add(uu[:], ee[:], 1.0)
        s0 = stile(shp, "s0"); nc.scalar.activation(s0[:], uu[:], AF.Ln)
        e1 = stile(shp, "e1"); nc.scalar.activation(e1[:], s0[:], AF.Exp)
        l1 = stile(shp, "l1"); nc.scalar.activation(l1[:], e1[:], AF.Ln)
        r1 = stile(shp, "r1"); nc.vector.tensor_sub(r1[:], s0[:], l1[:])
        t1 = stile(shp, "t1"); nc.vector.tensor_mul(t1[:], e1[:], r1[:])
        e1p = stile(shp, "e1p"); nc.vector.tensor_add(e1p[:], e1[:], t1[:])
        re1 = stile(shp, "re1"); nc.vector.reciprocal(re1[:], e1p[:])
        dd = stile(shp, "dd"); nc.vector.tensor_mul(dd[:], uu[:], re1[:])
        dm = stile(shp, "dm"); nc.vector.tensor_scalar_add(dm[:], dd[:], -1.0)
        sp = stile(shp, "sp"); nc.vector.tensor_add(sp[:], s0[:], dm[:])

        rn_sb = stile(shp, "rn_sb")
        nc.gpsimd.dma_start(
            rn_sb[:],
            A.rn[c0 * P:(c0 + RC) * P, :].rearrange("(t p) e -> p t e", p=P))
        noise = stile(shp, "noise"); nc.vector.tensor_mul(noise[:], rn_sb[:], sp[:])
        noisy = stile(shp, "noisy"); nc.vector.tensor_add(noisy[:], lg, noise[:])

        m1 = stile([P, RC], "m1")
        nc.vector.tensor_reduce(m1[:], noisy[:], axis=AX.X, op=OP.max)
        m1b = m1[:, :, None].broadcast_to(shp)
        eq = stile(shp, "eq")
        nc.vector.tensor_tensor(eq[:], noisy[:], m1b, op=OP.is_equal)
        big = stile(shp, "big"); nc.vector.tensor_scalar_mul(big[:], eq[:], 1e30)
        noisy2 = stile(shp, "noisy2"); nc.vector.tensor_sub(noisy2[:], noisy[:], big[:])
        m2 = stile([P, RC], "m2")
        nc.vector.tensor_reduce(m2[:], noisy2[:], axis=AX.X, op=OP.max)
        m2b = m2[:, :, None].broadcast_to(shp)
        ge = stile(shp, "ge")
        nc.vector.tensor_tensor(ge[:], noisy[:], m2b, op=OP.is_ge)
        shd = stile(shp, "shd"); nc.vector.tensor_sub(shd[:], noisy[:], m1b)
        ex = stile(shp, "ex"); nc.scalar.activation(ex[:], shd[:], AF.Exp)
        gg = stile(shp, "gg"); nc.vector.tensor_mul(gg[:], ex[:], ge[:])
        den = stile([P, RC], "den")
        nc.vector.tensor_reduce(den[:], gg[:], axis=AX.X, op=OP.add)
        rden = stile([P, RC], "rden")
        nc.vector.reciprocal(rden[:], den[:])
        gate8 = stile(shp, "gate8")
        nc.vector.tensor_tensor(gate8[:], gg[:],
                                rden[:, :, None].broadcast_to(shp), op=OP.mult)
        gsel = stile(shp, "gsel")
        nc.vector.tensor_tensor(gsel[:], gate8[:],
                                esel_sb[:, None, :].broadcast_to(shp), op=OP.mult)
        nc.vector.tensor_reduce(gate[:, c0:c0 + RC], gsel[:], axis=AX.X, op=OP.add)
        msel = stile(shp, "msel")
        nc.vector.tensor_tensor(msel[:], ge[:],
                                esel_sb[:, None, :].broadcast_to(shp), op=OP.mult)
        nc.vector.tensor_reduce(mask[:, c0:c0 + RC], msel[:], axis=AX.X, op=OP.add)

    # ---- compaction: slot = prefix(mask); unselected -> cap ----
    cntp = rps([TT, 1], "cntp")
    nc.tensor.matmul(cntp[:], mask[:], onescol_sb[:], start=True, stop=True)
    cnt_sb = stile([TT, 1], "cnt_sb")
    nc.scalar.activation(cnt_sb[:], cntp[:], AF.Copy)
    ecsp = rps([1, TT], "ecsp")
    nc.tensor.matmul(ecsp[:], cnt_sb[:], triu32_sb[:], start=True, stop=True)
    ecs_row = stile([1, TT], "ecs_row")
    nc.scalar.activation(ecs_row[:], ecsp[:], AF.Copy)
    totp = rps([1, 1], "totp")
    nc.tensor.matmul(totp[:], cnt_sb[:], onescol_sb[:TT, :], start=True, stop=True)
    tot_sb = stile([1, 1], "tot_sb")
    nc.scalar.activation(tot_sb[:], totp[:], AF.Copy)
    nc.sync.dma_start(A.cnt_t[:], tot_sb[:])

    posp = rps([P, TT], "posp")
    nc.tensor.matmul(posp[:], triu128_sb[:], mask[:], start=True, stop=False)
    nc.tensor.matmul(posp[:], ones32_sb[:1, :], ecs_row[:1, :],
                     start=False, stop=True)
    pos = stile([P, TT], "pos")
    nc.scalar.activation(pos[:], posp[:], AF.Copy)
    # pos_final = pos*mask + (1-mask)*cap
    pm_a = stile([P, TT], "pm_a"); nc.vector.tensor_mul(pm_a[:], pos[:], mask[:])
    pm_b = stile([P, TT], "pm_b")
    nc.vector.tensor_scalar_mul(pm_b[:], mask[:], float(cap))
    pm_c = stile([P, TT], "pm_c"); nc.vector.tensor_sub(pm_c[:], pm_a[:], pm_b[:])
    pm = stile([P, TT], "pm")
    nc.vector.tensor_scalar_add(pm[:], pm_c[:], float(cap))
    pos_i = stile([P, TT], "pos_i", I32)
    nc.vector.tensor_copy(pos_i[:], pm[:])

    # ---- slot tables: scatter (tokid, gate) by slot; OOB rows dropped --
    pay = stile([P, TT, 2], "pay")
    nc.sync.dma_start(pay[:], A.payt[:])
    nc.vector.tensor_copy(pay[:, :, 1], gate[:])
    for tt in range(TT):
        nc.gpsimd.indirect_dma_start(
            out=A.scat[:],
            out_offset=IndirectOffsetOnAxis(ap=pos_i[:, tt:tt + 1], axis=0),
            in_=pay[:, tt, :],
            in_offset=None,
            bounds_check=cap - 1,
            oob_is_err=False)
    scat_sb = stile([P, NTC, 2], "scat_sb")
    nc.gpsimd.dma_start(scat_sb[:],
                        A.scat.rearrange("(st p) c -> p st c", p=P))
    idx_i = stile([P, NTC], "idx_i", I32)
    nc.vector.tensor_copy(idx_i[:], scat_sb[:, :, 0])

    # ---- gather x rows (bf16) per half, transpose to xgT [d, slot] ----
    def emit_gather(h):
        st0, nst = HALVES[h]
        xgT = xgpool.tile([P, DS, nst * P], BF16, name=f"xgT{h}",
                          tag=f"xgT{h}")
        for sl in range(nst):
            st = st0 + sl
            xg = gpool.tile([P, D], BF16, name="xg", tag="xg")
            nc.gpsimd.indirect_dma_start(
                out=xg[:], in_=A.xrowsb[:],
                in_offset=IndirectOffsetOnAxis(ap=idx_i[:, st:st + 1], axis=0),
                out_offset=None,
                bounds_check=n_tok - 1,
                oob_is_err=False)
            for dp in range(DS):
                tps = rps([P, P], "tpsg", dt=BF16)
                nc.tensor.transpose(tps[:], xg[:, dp * P:(dp + 1) * P],
                                    id128b_sb[:])
                nc.scalar.activation(xgT[:, dp, sl * P:(sl + 1) * P], tps[:],
                                     AF.Copy)
        return xgT

    # ---- FFN building blocks (bf16 stationaries/moving, fp32 psum) ----
    def gemm1(xsrc, nb, w1b_t, hT_t, bias_sb, bias_off, nsub, chunk, relu_eng):
        ch = _chunks(nb, chunk)
        for hs in range(nsub):
            pss = [psum.tile([P, cw], F32, name="ps_g1", tag="ps")
                   for (_, cw) in ch]
            for ds in range(DS):
                for ci, (no, nw) in enumerate(ch):
                    nc.tensor.matmul(
                        pss[ci][:], w1b_t[:, ds, hs * P:(hs + 1) * P],
                        xsrc[:, ds, no:no + nw],
                        start=(ds == 0), stop=(ds == DS - 1))
            for ci, (no, nw) in enumerate(ch):
                bcol = bias_sb[:, bias_off + hs:bias_off + hs + 1]
                if relu_eng == "scalar":
                    nc.scalar.activation(
                        hT_t[:, hs, no:no + nw], pss[ci][:], AF.Relu,
                        bias=bcol)
                else:
                    nc.vector.tensor_scalar(
                        hT_t[:, hs, no:no + nw], pss[ci][:], bcol, 0.0,
                        op0=OP.add, op1=OP.max)

    # ---- routed FFN halves ----
    def emit_routed_half(h, xgT):
        st0, nst = HALVES[h]
        ntok_h = nst * P
        hT_h = hrpool.tile([P, H // P, ntok_h], BF16, name=f"hTr{h}",
                           tag="hTr")
        for hb in range(H // 512):
            w1b = w1pool.tile([P, DS, 512], BF16, name="w1b", tag="w1b")
            for (ho, hw) in _chunks(512, 256):
                nc.sync.dma_start(
                    w1b[:, :, ho:ho + hw],
                    A.w1[:, hb * 512 + ho:hb * 512 + ho + hw].rearrange(
                        "(ds p) h -> p ds h", p=P))
            gemm1(xgT, ntok_h, w1b, hT_h[:, hb * 4:(hb + 1) * 4, :],
                  b1_sb, hb * 4, 4, chunk=320 if ntok_h == 640 else 512,
                  relu_eng="scalar")
        for dh in range(2):
            w2h = w2rpool.tile([P, H // P, 512], BF16, name="w2h", tag="w2h")
            for (ho, hw) in _chunks(H, 1024):
                nc.sync.dma_start(
                    w2h[:, ho // P:(ho + hw) // P, :],
                    A.w2[ho:ho + hw, dh * 512:(dh + 1) * 512].rearrange(
                        "(hs p) d -> p hs d", p=P))
            for tl in range(nst):
                ps = psum.tile([P, 512], F32, name="ps_g2", tag="ps")
                for hsb in range(H // P):
                    nc.tensor.matmul(
                        ps[:], hT_h[:, hsb, tl * P:(tl + 1) * P],
                        w2h[:, hsb, :],
                        start=(hsb == 0), stop=(hsb == H // P - 1))
                tt = st0 + tl
                ystg = ypool.tile([P, 512], F32, name="ystg", tag="ystg")
                nc.scalar.activation(ystg[:], ps[:], AF.Copy,
                                     scale=scat_sb[:, tt, 1:2])
                nc.sync.dma_start(
                    A.out_rt[tt * P:(tt + 1) * P, dh * 512:(dh + 1) * 512],
                    ystg[:])

    # ---- shared-expert weights: loaded once, reused by all blocks ----
    sw1b = swpool.tile([P, DS, HS], BF16, name="sw1b", tag="sw1b")
    for (ho_, hw_) in _chunks(HS, 256):
        nc.sync.dma_start(
            sw1b[:, :, ho_:ho_ + hw_],
            A.sw1[:, ho_:ho_ + hw_].rearrange("(ds p) h -> p ds h", p=P))
    sw2b = swpool.tile([P, HS // P, D], BF16, name="sw2b", tag="sw2b")
    for (do_, dw_) in _chunks(D, 256):
        nc.sync.dma_start(
            sw2b[:, :, do_:do_ + dw_],
            A.sw2[:, do_:do_ + dw_].rearrange("(hs p) d -> p hs d", p=P))

    # ---- shared-expert block ----
    def shared_block(b):
        tok0 = b * NB
        NT = NB // P
        xb = xpool.tile([P, DS, NB], BF16, name="xb", tag="xb")
        for (no_, nw_) in (_chunks(NB) if b == 0 else [(0, NB)]):
            nc.sync.dma_start(
                xb[:, :, no_:no_ + nw_],
                A.xTb[:, tok0 + no_:tok0 + no_ + nw_].rearrange(
                    "(ds p) t -> p ds t", p=P))
        hTs = hspool.tile([P, HS // P, NB], BF16, name="hTs", tag="hTs")
        gemm1(xb, NB, sw1b, hTs, sb1_sb, 0, HS // P, chunk=512,
              relu_eng="vector")
        for tt in range(NT):
            pss = [psum.tile([P, 512], F32, name="ps_s2", tag="ps")
                   for _ in range(2)]
            for hsb in range(HS // P):
                for ci in range(2):
                    nc.tensor.matmul(
                        pss[ci][:], hTs[:, hsb, tt * P:(tt + 1) * P],
                        sw2b[:, hsb, ci * 512:(ci + 1) * 512],
                        start=(hsb == 0), stop=(hsb == HS // P - 1))
            for ci in range(2):
                ystg = ypool.tile([P, 512], F32, name="ystg", tag="ystg")
                nc.vector.tensor_copy(ystg[:], pss[ci][:])
                nc.sync.dma_start(
                    A.out_sh[tok0 + tt * P:tok0 + (tt + 1) * P,
                             ci * 512:(ci + 1) * 512],
                    ystg[:])

    # ---- emission order: router machinery first (critical path), shared
    # blocks fill PE gaps, routed halves as soon as gathers land ----
    xgT_A = emit_gather(0)
    xgT_B = emit_gather(1)
    shared_block(0)
    emit_routed_half(0, xgT_A)
    shared_block(1)
    emit_routed_half(1, xgT_B)
    shared_block(2)
    shared_block(3)


# ---------------- host side ----------------

_NC_CACHE = {}


def _get_nc(n_tok, cap):
    key = (n_tok, cap)
    if key not in _NC_CACHE:
        _NC_CACHE[key] = build_nc(n_tok, cap)
    return _NC_CACHE[key]


def make_in_maps(n_tok, cap, x, router_noise, Wr, br, Wn, bn, rW1, rb1, rW2,
                 rb2, sW1, sb1, sW2, sb2):
    TT = n_tok // P
    BF = ml_dtypes.bfloat16
    xf = np.ascontiguousarray(x.reshape(n_tok, D))
    xT = np.ascontiguousarray(xf.T)
    xTb = xT.astype(BF)
    xrowsb = xf.astype(BF)
    rnf = np.ascontiguousarray(router_noise.reshape(n_tok, E)).astype(np.float32)
    wrn = np.ascontiguousarray(np.concatenate([Wr, Wn], axis=1)).astype(np.float32)
    brbn = np.concatenate([br, bn]).reshape(2 * E, 1).astype(np.float32)
    payt = np.zeros((P, TT, 2), np.float32)
    payt[:, :, 0] = (np.arange(TT)[None, :] * P + np.arange(P)[:, None])

    in_maps = []
    for c in range(N_CORES):
        se, hsl = c // 4, (c % 4) * HS
        esel = np.zeros((P, E), np.float32)
        esel[:, c] = 1.0
        in_maps.append({
            "xT": xT,
            "xTb": xTb,
            "xrowsb": xrowsb,
            "rn": rnf,
            "wrn": wrn,
            "brbn": brbn,
            "esel": esel,
            "ones32": np.ones((1, P), np.float32),
            "onescol": np.ones((P, 1), np.float32),
            "triu128": np.triu(np.ones((P, P), np.float32), 1),
            "triu32": np.triu(np.ones((TT, TT), np.float32), 1),
            "id16": np.eye(16, dtype=np.float32),
            "id128b": np.eye(P, dtype=BF),
            "payt": payt,
            "w1": np.ascontiguousarray(rW1[c]).astype(BF),
            "b1": np.ascontiguousarray(rb1[c].reshape(H // P, P).T),
            "w2": np.ascontiguousarray(rW2[c]).astype(BF),
            "sw1": np.ascontiguousarray(sW1[se][:, hsl:hsl + HS]).astype(BF),
            "sb1": np.ascontiguousarray(
                sb1[se][hsl:hsl + HS].reshape(HS // P, P).T),
            "sw2": np.ascontiguousarray(sW2[se][hsl:hsl + HS, :]).astype(BF),
        })
    return in_maps


def combine(x, results, n_tok, cap, rb2, sb2):
    acc = x.reshape(n_tok, D).astype(np.float32).copy()
    acc += sb2.sum(axis=0).astype(np.float32)
    for c in range(N_CORES):
        acc += results[c]["out_sh"]
    for c in range(N_CORES):
        n = int(round(float(results[c]["cnt_t"][0, 0])))
        assert n <= cap, f"core {c}: count {n} exceeds capacity {cap}"
        sc = results[c]["scat"]
        idx = np.rint(sc[:n, 0]).astype(np.int64)
        g = sc[:n, 1:2]
        acc[idx] += results[c]["out_rt"][:n] + g * rb2[c][None, :]
    return acc


def kernel(x, router_noise, topk, Wr, br, Wn, bn, rW1, rb1, rW2, rb2,
           sW1, sb1, sW2, sb2, _trace=False):
    assert int(topk) == 2
    x = np.asarray(x, np.float32)
    B, T, Dx = x.shape
    n_tok = B * T
    nc = _get_nc(n_tok, CAP)
    in_maps = make_in_maps(
        n_tok, CAP, x, np.asarray(router_noise, np.float32),
        np.asarray(Wr, np.float32), np.asarray(br, np.float32),
        np.asarray(Wn, np.float32), np.asarray(bn, np.float32),
        np.asarray(rW1, np.float32), np.asarray(rb1, np.float32),
        np.asarray(rW2, np.float32), np.asarray(rb2, np.float32),
        np.asarray(sW1, np.float32), np.asarray(sb1, np.float32),
        np.asarray(sW2, np.float32), np.asarray(sb2, np.float32))
    res = run_bass_kernel_spmd(nc, in_maps, core_ids=list(range(N_CORES)),
                               trace=_trace)
    out = combine(x, res.results, n_tok, CAP,
                  np.asarray(rb2, np.float32),
                  np.asarray(sb2, np.float32)).reshape(B, T, Dx)
    if _trace:
        return out, res
    return out


# revision 13
# speedup vs baseline: 1.2850x; 1.0696x over previous
"""DeepSeekMoE forward on 8 TRN2 cores — v3.

Sharding: routed expert c -> core c; shared experts 8-way H-sliced;
router replicated. vs v2:

  - scatter/gather batched into single indirect DMAs (offset AP carries
    all 4096/1152 indices) — v2 spent ~130us of critical path on 41
    separate descgen ops spaced ~3us apart by sequencer latency.
  - router softplus/top-2 chain runs as ONE full-width island
    ([P,32,8] tiles) instead of 4 sequential 33-op islands: latency
    ~22us instead of ~91us; in-place buffer reuse keeps SBUF flat.
  - separate PSUM + staging pools for shared vs routed phases: v2's
    shared pool rotation serialized shared blocks 2/3 behind the
    routed halves (88us PE idle while waiting on the gather).
  - shared blocks shrunk to 512 tokens (8 blocks) for finer-grained
    gap filling and smaller xb/hTs footprints.
"""

import sys
from contextlib import ExitStack

if "/opt/trn_rl_repo" not in sys.path:
    sys.path.insert(0, "/opt/trn_rl_repo")

import numpy as np
import ml_dtypes

import concourse.bass as bass
import concourse.mybir as mybir
import concourse.tile as tile
from concourse import bacc
from concourse.bass import IndirectOffsetOnAxis
from concourse.bass_utils import run_bass_kernel_spmd

F32 = mybir.dt.float32
BF16 = mybir.dt.bfloat16
I32 = mybir.dt.int32
AF = mybir.ActivationFunctionType
OP = mybir.AluOpType
AX = mybir.AxisListType

N_CORES = 8
D = 1024
H = 4096
HS = 1024
E = 8
P = 128

NB = 512           # shared-expert token block
RXC = 128          # router moving-chunk (tokens)
CAP = 1152
HALVES = [(0, 5), (5, 4)]   # (start st-tile, n st-tiles) of cap/128=9


def _chunks(n, step=512):
    out, o = [], 0
    while o < n:
        out.append((o, min(step, n - o)))
        o += step
    return out


def build_nc(n_tok: int, cap: int, num_devices: int = N_CORES):
    assert n_tok % NB == 0 and cap % P == 0
    nc = bacc.Bacc("TRN2", target_bir_lowering=False, debug=False,
                   num_devices=num_devices)
    aps = {}

    def dram(name, shape, dt, kind="ExternalInput"):
        aps[name] = nc.dram_tensor(name, shape, dt, kind=kind).ap()

    TT = n_tok // P
    dram("xT", [D, n_tok], F32)          # router moving operand
    dram("xTb", [D, n_tok], BF16)        # shared-expert moving operand
    dram("xrowsb", [n_tok, D], BF16)     # gather source
    dram("rn", [n_tok, E], F32)
    dram("wrn", [D, 2 * E], F32)
    dram("brbn", [2 * E, 1], F32)
    dram("esel", [P, E], F32)
    dram("ones32", [1, P], F32)
    dram("onescol", [P, 1], F32)
    dram("triu128", [P, P], F32)         # [j,i]=1 if j<i
    dram("triu32", [TT, TT], F32)
    dram("id16", [16, 16], F32)
    dram("id128b", [P, P], BF16)
    dram("payt", [P, TT, 2], F32)        # [:,tt,0]=tt*128+p, [:,tt,1]=0
    dram("w1", [D, H], BF16)
    dram("b1", [P, H // P], F32)
    dram("w2", [H, D], BF16)
    dram("sw1", [D, HS], BF16)
    dram("sb1", [P, HS // P], F32)
    dram("sw2", [HS, D], BF16)
    dram("scat", [cap, 2], F32, kind="ExternalOutput")
    dram("out_sh", [n_tok, D], F32, kind="ExternalOutput")
    dram("out_rt", [cap, D], F32, kind="ExternalOutput")
    dram("cnt_t", [1, 1], F32, kind="ExternalOutput")

    with tile.TileContext(nc) as tc:
        with ExitStack() as es:
            _emit(es, tc, nc, aps, n_tok, cap)
    nc.compile()
    return nc


def _emit(es, tc, nc, aps, n_tok, cap):
    TT = n_tok // P
    DS = D // P
    NTC = cap // P

    A = type("A", (), aps)

    cpool = es.enter_context(tc.tile_pool(name="const", bufs=1))
    rxpool = es.enter_context(tc.tile_pool(name="rx", bufs=2))
    spool = es.enter_context(tc.tile_pool(name="rscratch", bufs=1))
    rpsum = es.enter_context(tc.tile_pool(name="rpsum", bufs=2, space="PSUM"))
    xpool = es.enter_context(tc.tile_pool(name="xb", bufs=1))
    xgpool = es.enter_context(tc.tile_pool(name="xgt", bufs=1))
    w1pool = es.enter_context(tc.tile_pool(name="w1b", bufs=2))
    swpool = es.enter_context(tc.tile_pool(name="swb", bufs=1))
    w2rpool = es.enter_context(tc.tile_pool(name="w2r", bufs=1))
    hspool = es.enter_context(tc.tile_pool(name="hTs", bufs=1))
    hrpool = es.enter_context(tc.tile_pool(name="hTr", bufs=1))
    yspool = es.enter_context(tc.tile_pool(name="ysgS", bufs=2))
    yrpool = es.enter_context(tc.tile_pool(name="ysgR", bufs=2))
    psumS = es.enter_context(tc.tile_pool(name="psumS", bufs=3, space="PSUM"))
    psumR = es.enter_context(tc.tile_pool(name="psumR", bufs=3, space="PSUM"))

    def ctile(shape, dt, name):
        return cpool.tile(shape, dt, name=name, tag=name)

    def stile(shape, name, dt=F32, bufs=None):
        return spool.tile(shape, dt, name=name, tag=name, bufs=bufs)

    def rps(shape, name, dt=F32):
        return rpsum.tile(shape, dt, name=name, tag="rps")

    def load_const(name, shape, dt):
        t = ctile(shape, dt, name + "_sb")
        nc.sync.dma_start(t[:], aps[name][:])
        return t

    # ---- constants ----
    wrn_sb = ctile([P, DS, 2 * E], F32, "wrn_sb")
    nc.sync.dma_start(wrn_sb[:], A.wrn.rearrange("(ds p) e -> p ds e", p=P))
    brbn_sb = load_const("brbn", [2 * E, 1], F32)
    esel_sb = load_const("esel", [P, E], F32)
    ones32_sb = load_const("ones32", [1, P], F32)
    onescol_sb = load_const("onescol", [P, 1], F32)
    triu128_sb = load_const("triu128", [P, P], F32)
    triu32_sb = load_const("triu32", [TT, TT], F32)
    id16_sb = load_const("id16", [16, 16], F32)
    id128b_sb = load_const("id128b", [P, P], BF16)
    b1_sb = load_const("b1", [P, H // P], F32)
    sb1_sb = load_const("sb1", [P, HS // P], F32)

    # ---- router matmuls: noisy = [logits | pre-softplus] -> lgnl ----
    lgnl = stile([P, TT, 2 * E], "lgnl")
    for rb in range(n_tok // RXC):
        xr = rxpool.tile([P, DS, RXC], F32, name="xr", tag="xr")
        nc.gpsimd.dma_start(
            xr[:],
            A.xT[:, rb * RXC:(rb + 1) * RXC].rearrange(
                "(ds p) t -> p ds t", p=P))
        ps_r = rps([2 * E, RXC], "ps_r")
        for ds in range(DS):
            nc.tensor.matmul(ps_r[:], wrn_sb[:, ds, :], xr[:, ds, :],
                             start=(ds == 0), stop=(ds == DS - 1))
        lgch = stile([2 * E, RXC], "lgch", bufs=2)
        nc.scalar.activation(lgch[:], ps_r[:], AF.Identity, bias=brbn_sb[:])
        for k in range(RXC // P):
            tps = rps([P, 2 * E], "tps_r")
            nc.tensor.transpose(tps[:], lgch[:, k * P:(k + 1) * P], id16_sb[:])
            tt = (rb * RXC) // P + k
            nc.scalar.activation(lgnl[:, tt, :], tps[:], AF.Copy)

    # ---- router chain, one full-width island; B0-B3 reused in place ----
    shp = [P, TT, E]
    lg = lgnl[:, :, 0:E]
    nl = lgnl[:, :, E:2 * E]
    B0 = stile(shp, "B0"); B1 = stile(shp, "B1")
    B2 = stile(shp, "B2"); B3 = stile(shp, "B3")
    rn_sb = stile(shp, "rn_sb")
    nc.gpsimd.dma_start(rn_sb[:],
                        A.rn.rearrange("(t p) e -> p t e", p=P))
    V, S = nc.vector, nc.scalar
    # compensated softplus: sp = s0 + (uu / exp_c(s0) - 1)
    S.activation(B0[:], nl, AF.Exp)                       # B0 = e0
    S.activation(B1[:], B0[:], AF.Ln)                     # B1 = l0
    V.tensor_tensor(B1[:], nl, B1[:], op=OP.subtract)     # B1 = r0
    V.tensor_tensor(B1[:], B0[:], B1[:], op=OP.mult)      # B1 = t0
    V.tensor_tensor(B1[:], B0[:], B1[:], op=OP.add)       # B1 = ee
    V.tensor_scalar_add(B1[:], B1[:], 1.0)                # B1 = uu
    S.activation(B0[:], B1[:], AF.Ln)                     # B0 = s0
    S.activation(B2[:], B0[:], AF.Exp)                    # B2 = e1
    S.activation(B3[:], B2[:], AF.Ln)                     # B3 = l1
    V.tensor_tensor(B3[:], B0[:], B3[:], op=OP.subtract)  # B3 = r1
    V.tensor_tensor(B3[:], B2[:], B3[:], op=OP.mult)      # B3 = t1
    V.tensor_tensor(B3[:], B2[:], B3[:], op=OP.add)       # B3 = e1p
    V.reciprocal(B2[:], B3[:])                            # B2 = re1
    V.tensor_tensor(B2[:], B1[:], B2[:], op=OP.mult)      # B2 = dd
    V.tensor_scalar_add(B2[:], B2[:], -1.0)               # B2 = dm
    V.tensor_tensor(B2[:], B0[:], B2[:], op=OP.add)       # B2 = sp
    V.tensor_tensor(B2[:], rn_sb[:], B2[:], op=OP.mult)   # B2 = noise
    V.tensor_tensor(B2[:], lg, B2[:], op=OP.add)          # B2 = noisy
    m1 = stile([P, TT], "m1")
    V.tensor_reduce(m1[:], B2[:], axis=AX.X, op=OP.max)
    m1b = m1[:, :, None].broadcast_to(shp)
    V.tensor_tensor(B0[:], B2[:], m1b, op=OP.is_equal)    # B0 = eq(top1)
    V.tensor_scalar_mul(B0[:], B0[:], 1e30)
    V.tensor_tensor(B0[:], B2[:], B0[:], op=OP.subtract)  # B0 = noisy2
    m2 = stile([P, TT], "m2")
    V.tensor_reduce(m2[:], B0[:], axis=AX.X, op=OP.max)
    m2b = m2[:, :, None].broadcast_to(shp)
    V.tensor_tensor(B0[:], B2[:], m2b, op=OP.is_ge)       # B0 = ge (top2 sel)
    V.tensor_tensor(B1[:], B2[:], m1b, op=OP.subtract)    # B1 = shd
    S.activation(B1[:], B1[:], AF.Exp)                    # B1 = ex
    V.tensor_tensor(B1[:], B1[:], B0[:], op=OP.mult)      # B1 = gg
    den = stile([P, TT], "den")
    V.tensor_reduce(den[:], B1[:], axis=AX.X, op=OP.add)
    rden = stile([P, TT], "rden")
    V.reciprocal(rden[:], den[:])
    V.tensor_tensor(B1[:], B1[:], rden[:, :, None].broadcast_to(shp),
                    op=OP.mult)                           # B1 = gate8
    eselb = esel_sb[:, None, :].broadcast_to(shp)
    V.tensor_tensor(B3[:], B1[:], eselb, op=OP.mult)
    gate = stile([P, TT], "gate")
    V.tensor_reduce(gate[:], B3[:], axis=AX.X, op=OP.add)
    V.tensor_tensor(B3[:], B0[:], eselb, op=OP.mult)
    mask = stile([P, TT], "mask")
    V.tensor_reduce(mask[:], B3[:], axis=AX.X, op=OP.add)

    # ---- compaction: slot = prefix(mask); unselected -> cap ----
    cntp = rps([TT, 1], "cntp")
    nc.tensor.matmul(cntp[:], mask[:], onescol_sb[:], start=True, stop=True)
    cnt_sb = stile([TT, 1], "cnt_sb")
    nc.scalar.activation(cnt_sb[:], cntp[:], AF.Copy)
    ecsp = rps([1, TT], "ecsp")
    nc.tensor.matmul(ecsp[:], cnt_sb[:], triu32_sb[:], start=True, stop=True)
    ecs_row = stile([1, TT], "ecs_row")
    nc.scalar.activation(ecs_row[:], ecsp[:], AF.Copy)
    totp = rps([1, 1], "totp")
    nc.tensor.matmul(totp[:], cnt_sb[:], onescol_sb[:TT, :], start=True, stop=True)
    tot_sb = stile([1, 1], "tot_sb")
    nc.scalar.activation(tot_sb[:], totp[:], AF.Copy)
    nc.sync.dma_start(A.cnt_t[:], tot_sb[:])

    posp = rps([P, TT], "posp")
    nc.tensor.matmul(posp[:], triu128_sb[:], mask[:], start=True, stop=False)
    nc.tensor.matmul(posp[:], ones32_sb[:1, :], ecs_row[:1, :],
                     start=False, stop=True)
    pos = stile([P, TT], "pos")
    nc.scalar.activation(pos[:], posp[:], AF.Copy)
    # pos_final = pos*mask + (1-mask)*cap, in place
    V.tensor_tensor(pos[:], pos[:], mask[:], op=OP.mult)
    pm_b = stile([P, TT], "pm_b")
    V.tensor_scalar(pm_b[:], mask[:], -float(cap), float(cap),
                    op0=OP.mult, op1=OP.add)              # cap*(1-mask)
    V.tensor_tensor(pos[:], pos[:], pm_b[:], op=OP.add)
    pos_i = stile([P, TT], "pos_i", I32)
    V.tensor_copy(pos_i[:], pos[:])

    # ---- slot tables: ONE batched scatter; OOB (pos=cap) dropped ----
    pay = stile([P, TT, 2], "pay")
    nc.sync.dma_start(pay[:], A.payt[:])
    V.tensor_copy(pay[:, :, 1], gate[:])
    for tt in range(TT):
        nc.gpsimd.indirect_dma_start(
            out=A.scat[:],
            out_offset=IndirectOffsetOnAxis(ap=pos_i[:, tt:tt + 1], axis=0),
            in_=pay[:, tt, :],
            in_offset=None,
            bounds_check=cap - 1,
            oob_is_err=False)
    scat_sb = stile([P, NTC, 2], "scat_sb")
    nc.gpsimd.dma_start(scat_sb[:],
                        A.scat.rearrange("(st p) c -> p st c", p=P))
    idx_i = stile([P, NTC], "idx_i", I32)
    V.tensor_copy(idx_i[:], scat_sb[:, :, 0])

    # ---- batched gather per half + transpose to xgT [d, slot] ----
    def emit_gather(h):
        st0, nst = HALVES[h]
        xga = xgpool.tile([P, nst, D], BF16, name=f"xga{h}", tag=f"xga{h}")
        for sl in range(nst):
            nc.gpsimd.indirect_dma_start(
                out=xga[:, sl, :], in_=A.xrowsb[:],
                in_offset=IndirectOffsetOnAxis(
                    ap=idx_i[:, st0 + sl:st0 + sl + 1], axis=0),
                out_offset=None,
                bounds_check=n_tok - 1,
                oob_is_err=False)
        xgT = xgpool.tile([P, DS, nst * P], BF16, name=f"xgT{h}",
                          tag=f"xgT{h}")
        for sl in range(nst):
            for dp in range(DS):
                tps = rps([P, P], "tpsg", dt=BF16)
                nc.tensor.transpose(tps[:], xga[:, sl, dp * P:(dp + 1) * P],
                                    id128b_sb[:])
                nc.scalar.activation(xgT[:, dp, sl * P:(sl + 1) * P], tps[:],
                                     AF.Copy)
        return xgT

    # ---- FFN building blocks (bf16 operands, fp32 psum) ----
    def gemm1(xsrc, nb, w1b_t, hT_t, bias_sb, bias_off, nsub, chunk, relu_eng,
              pp):
        ch = _chunks(nb, chunk)
        for hs in range(nsub):
            pss = [pp.tile([P, cw], F32, name="ps_g1", tag="ps")
                   for (_, cw) in ch]
            for ds in range(DS):
                for ci, (no, nw) in enumerate(ch):
                    nc.tensor.matmul(
                        pss[ci][:], w1b_t[:, ds, hs * P:(hs + 1) * P],
                        xsrc[:, ds, no:no + nw],
                        start=(ds == 0), stop=(ds == DS - 1))
            for ci, (no, nw) in enumerate(ch):
                bcol = bias_sb[:, bias_off + hs:bias_off + hs + 1]
                if relu_eng == "scalar":
                    nc.scalar.activation(
                        hT_t[:, hs, no:no + nw], pss[ci][:], AF.Relu,
                        bias=bcol)
                else:
                    nc.vector.tensor_scalar(
                        hT_t[:, hs, no:no + nw], pss[ci][:], bcol, 0.0,
                        op0=OP.add, op1=OP.max)

    # ---- routed FFN halves ----
    def emit_routed_half(h, xgT):
        st0, nst = HALVES[h]
        ntok_h = nst * P
        hT_h = hrpool.tile([P, H // P, ntok_h], BF16, name=f"hTr{h}",
                           tag="hTr")
        for hb in range(H // 512):
            w1b = w1pool.tile([P, DS, 512], BF16, name="w1b", tag="w1b")
            for (ho, hw) in _chunks(512, 256):
                nc.sync.dma_start(
                    w1b[:, :, ho:ho + hw],
                    A.w1[:, hb * 512 + ho:hb * 512 + ho + hw].rearrange(
                        "(ds p) h -> p ds h", p=P))
            gemm1(xgT, ntok_h, w1b, hT_h[:, hb * 4:(hb + 1) * 4, :],
                  b1_sb, hb * 4, 4, chunk=320 if ntok_h == 640 else 512,
                  relu_eng="scalar", pp=psumR)
        for dh in range(2):
            w2h = w2rpool.tile([P, H // P, 512], BF16, name="w2h", tag="w2h")
            for (ho, hw) in _chunks(H, 1024):
                nc.sync.dma_start(
                    w2h[:, ho // P:(ho + hw) // P, :],
                    A.w2[ho:ho + hw, dh * 512:(dh + 1) * 512].rearrange(
                        "(hs p) d -> p hs d", p=P))
            for tl in range(nst):
                ps = psumR.tile([P, 512], F32, name="ps_g2", tag="ps")
                for hsb in range(H // P):
                    nc.tensor.matmul(
                        ps[:], hT_h[:, hsb, tl * P:(tl + 1) * P],
                        w2h[:, hsb, :],
                        start=(hsb == 0), stop=(hsb == H // P - 1))
                tt = st0 + tl
                ystg = yrpool.tile([P, 512], F32, name="ystgR", tag="ystgR")
                nc.scalar.activation(ystg[:], ps[:], AF.Copy,
                                     scale=scat_sb[:, tt, 1:2])
                nc.sync.dma_start(
                    A.out_rt[tt * P:(tt + 1) * P, dh * 512:(dh + 1) * 512],
                    ystg[:])

    # ---- shared-expert weights: loaded once, reused by all blocks ----
    sw1b = swpool.tile([P, DS, HS], BF16, name="sw1b", tag="sw1b")
    for (ho_, hw_) in _chunks(HS, 256):
        nc.sync.dma_start(
            sw1b[:, :, ho_:ho_ + hw_],
            A.sw1[:, ho_:ho_ + hw_].rearrange("(ds p) h -> p ds h", p=P))
    sw2b = swpool.tile([P, HS // P, D], BF16, name="sw2b", tag="sw2b")
    for (do_, dw_) in _chunks(D, 256):
        nc.sync.dma_start(
            sw2b[:, :, do_:do_ + dw_],
            A.sw2[:, do_:do_ + dw_].rearrange("(hs p) d -> p hs d", p=P))

    # ---- shared-expert block (NB tokens) ----
    def shared_block(b):
        tok0 = b * NB
        NT = NB // P
        xb = xpool.tile([P, DS, NB], BF16, name="xb", tag="xb")
        for (no_, nw_) in (_chunks(NB, 256) if b == 0 else [(0, NB)]):
            nc.sync.dma_start(
                xb[:, :, no_:no_ + nw_],
                A.xTb[:, tok0 + no_:tok0 + no_ + nw_].rearrange(
                    "(ds p) t -> p ds t", p=P))
        hTs = hspool.tile([P, HS // P, NB], BF16, name="hTs", tag="hTs")
        gemm1(xb, NB, sw1b, hTs, sb1_sb, 0, HS // P, chunk=512,
              relu_eng="vector", pp=psumS)
        for tt in range(NT):
            pss = [psumS.tile([P, 512], F32, name="ps_s2", tag="ps")
                   for _ in range(2)]
            for hsb in range(HS // P):
                for ci in range(2):
                    nc.tensor.matmul(
                        pss[ci][:], hTs[:, hsb, tt * P:(tt + 1) * P],
                        sw2b[:, hsb, ci * 512:(ci + 1) * 512],
                        start=(hsb == 0), stop=(hsb == HS // P - 1))
            for ci in range(2):
                ystg = yspool.tile([P, 512], F32, name="ystgS", tag="ystgS")
                nc.scalar.activation(ystg[:], pss[ci][:], AF.Copy)
                nc.sync.dma_start(
                    A.out_sh[tok0 + tt * P:tok0 + (tt + 1) * P,
                             ci * 512:(ci + 1) * 512],
                    ystg[:])

    # ---- emission order: router machinery first (critical path); shared
    # blocks fill PE idle during chain/scatter; routed halves preempt ----
    xgT_A = emit_gather(0)
    xgT_B = emit_gather(1)
    shared_block(0)
    shared_block(1)
    shared_block(2)
    shared_block(3)
    emit_routed_half(0, xgT_A)
    shared_block(4)
    shared_block(5)
    emit_routed_half(1, xgT_B)
    shared_block(6)
    shared_block(7)


# ---------------- host side ----------------

_NC_CACHE = {}


def _get_nc(n_tok, cap):
    key = (n_tok, cap)
    if key not in _NC_CACHE:
        _NC_CACHE[key] = build_nc(n_tok, cap)
    return _NC_CACHE[key]


def make_in_maps(n_tok, cap, x, router_noise, Wr, br, Wn, bn, rW1, rb1, rW2,
                 rb2, sW1, sb1, sW2, sb2):
    TT = n_tok // P
    BF = ml_dtypes.bfloat16
    xf = np.ascontiguousarray(x.reshape(n_tok, D))
    xT = np.ascontiguousarray(xf.T)
    xTb = xT.astype(BF)
    xrowsb = xf.astype(BF)
    rnf = np.ascontiguousarray(router_noise.reshape(n_tok, E)).astype(np.float32)
    wrn = np.ascontiguousarray(np.concatenate([Wr, Wn], axis=1)).astype(np.float32)
    brbn = np.concatenate([br, bn]).reshape(2 * E, 1).astype(np.float32)
    payt = np.zeros((P, TT, 2), np.float32)
    payt[:, :, 0] = (np.arange(TT)[None, :] * P + np.arange(P)[:, None])

    in_maps = []
    for c in range(N_CORES):
        se, hsl = c // 4, (c % 4) * HS
        esel = np.zeros((P, E), np.float32)
        esel[:, c] = 1.0
        in_maps.append({
            "xT": xT,
            "xTb": xTb,
            "xrowsb": xrowsb,
            "rn": rnf,
            "wrn": wrn,
            "brbn": brbn,
            "esel": esel,
            "ones32": np.ones((1, P), np.float32),
            "onescol": np.ones((P, 1), np.float32),
            "triu128": np.triu(np.ones((P, P), np.float32), 1),
            "triu32": np.triu(np.ones((TT, TT), np.float32), 1),
            "id16": np.eye(16, dtype=np.float32),
            "id128b": np.eye(P, dtype=BF),
            "payt": payt,
            "w1": np.ascontiguousarray(rW1[c]).astype(BF),
            "b1": np.ascontiguousarray(rb1[c].reshape(H // P, P).T),
            "w2": np.ascontiguousarray(rW2[c]).astype(BF),
            "sw1": np.ascontiguousarray(sW1[se][:, hsl:hsl + HS]).astype(BF),
            "sb1": np.ascontiguousarray(
                sb1[se][hsl:hsl + HS].reshape(HS // P, P).T),
            "sw2": np.ascontiguousarray(sW2[se][hsl:hsl + HS, :]).astype(BF),
        })
    return in_maps


def combine(x, results, n_tok, cap, rb2, sb2):
    acc = x.reshape(n_tok, D).astype(np.float32).copy()
    acc += sb2.sum(axis=0).astype(np.float32)
    for c in range(N_CORES):
        acc += results[c]["out_sh"]
    for c in range(N_CORES):
        n = int(round(float(results[c]["cnt_t"][0, 0])))
        assert n <= cap, f"core {c}: count {n} exceeds capacity {cap}"
        sc = results[c]["scat"]
        idx = np.rint(sc[:n, 0]).astype(np.int64)
        g = sc[:n, 1:2]
        acc[idx] += results[c]["out_rt"][:n] + g * rb2[c][None, :]
    return acc


def kernel(x, router_noise, topk, Wr, br, Wn, bn, rW1, rb1, rW2, rb2,
           sW1, sb1, sW2, sb2, _trace=False):
    assert int(topk) == 2
    x = np.asarray(x, np.float32)
    B, T, Dx = x.shape
    n_tok = B * T
    nc = _get_nc(n_tok, CAP)
    in_maps = make_in_maps(
        n_tok, CAP, x, np.asarray(router_noise, np.float32),
        np.asarray(Wr, np.float32), np.asarray(br, np.float32),
        np.asarray(Wn, np.float32), np.asarray(bn, np.float32),
        np.asarray(rW1, np.float32), np.asarray(rb1, np.float32),
        np.asarray(rW2, np.float32), np.asarray(rb2, np.float32),
        np.asarray(sW1, np.float32), np.asarray(sb1, np.float32),
        np.asarray(sW2, np.float32), np.asarray(sb2, np.float32))
    res = run_bass_kernel_spmd(nc, in_maps, core_ids=list(range(N_CORES)),
                               trace=_trace)
    out = combine(x, res.results, n_tok, CAP,
                  np.asarray(rb2, np.float32),
                  np.asarray(sb2, np.float32)).reshape(B, T, Dx)
    if _trace:
        return out, res
    return out
